# revision 6
# baseline (speedup 1.0000x reference)
"""TRN2 Bass kernel for nn_Attn_63230508532520.

reference:
    proj = history @ W.T + b            # [S1, N]
    energies = out_state @ proj.T       # [S2, S1]
    out = softmax(energies, axis=-1)

Math used here:
    energies = out_state @ W @ history.T + (out_state @ b) 1^T
    The bias term is constant per row -> softmax-invariant -> dropped.
    G = out_state @ W (per-core slice), scores = G @ history.T, row softmax.

Sharding: out_state rows (S2=4096) split across 8 cores (512 rows each);
W and history replicated. history.T is fed as fp16 (PE runs fp16 at the
same 1-pass rate as fp32r; halves the dominant HBM stream 16->8 MB/core).
W/out_state stay fp32r (the PE rejects mixed 32/16-bit matmuls, so G is
stored fp16 for the scores matmul). Measured rel err ~1.1e-2 vs 2e-2 gate.

Per-core pipeline (inputs host-packed so each DMA is one large
contiguous-per-partition transfer on the sync HWDGE ring, strict FIFO
order ost/W interleaved -> ht pairs; PE cadence is 216ns/MM = roofline):
  Phase A: G.T [128m, 512s] per m-group = W-panel-stationary fp32r
           matmuls accumulated over n, PSUM evacuated to fp32r SBUF.
  Phase B part 1 (ht col-blocks 0..3): block-major over the first two
           ht pairs; per (block, s-chunk): 8 matmuls into a rotating
           PSUM bank, block max (DVE, negated), exp(x - max) from PSUM
           with accum_out row sums (ACT) into fp16 SBUF.
  Phase B part 2 (blocks 4..7): s-chunk-major so each chunk's softmax
           finalize + output store overlaps the next chunk's matmuls:
           global max over the 8 block maxes, factors
           f_ib = exp(m_ib - M_i)/S_i, scale of the 8 col-blocks split
           across DVE/ACT, stores on the (by then idle) sync ring; the
           last chunk stores quarter-wise across both HWDGE rings to
           shorten the tail.
Output fp16 (rounding 5e-4, far below the matmul noise); host upcasts.
"""
import os
import numpy as np
from contextlib import ExitStack

S2, S1, N = 4096, 4096, 1024
NCORES = 8
SC = S2 // NCORES          # 512 rows per core
NB_M = N // 128            # 8 contraction chunks
NB_I = SC // 128           # 4 s-chunks per core
NB_T = S1 // 512           # 8 t-blocks

_CACHE = {}


def _build():
    import concourse.bacc as bacc
    import concourse.mybir as mybir
    import concourse.tile as tile

    F32 = mybir.dt.float32
    F32R = mybir.dt.float32r
    F16 = mybir.dt.float16

    nc = bacc.Bacc()
    # host-packed layouts (see kernel() below)
    ost_r = nc.declare_dram_parameter("ost_r", [128, NB_M * SC], F32R, isOutput=False)
    w_r = nc.declare_dram_parameter("w_r", [128, NB_M * N], F32R, isOutput=False)
    ht_r = nc.declare_dram_parameter("ht_r", [128, NB_M * S1], F16, isOutput=False)
    probs = nc.declare_dram_parameter("probs", [SC, S1], F16, isOutput=True)

    with tile.TileContext(nc) as tc, ExitStack() as ctx:
        big = ctx.enter_context(tc.tile_pool(name="big", bufs=1))
        out_pool = ctx.enter_context(tc.tile_pool(name="outp", bufs=2))
        small = ctx.enter_context(tc.tile_pool(name="small", bufs=1))
        ps = ctx.enter_context(tc.tile_pool(name="ps", bufs=8, space="PSUM"))

        # ---- input loads: strict FIFO order on the sync ring, sized so
        # Phase A can start as early as possible and is never starved ----
        ost_sb = big.tile([128, NB_M * SC], F32R, tag="ost", name="ost")
        w_sb = big.tile([128, NB_M * N], F32R, tag="w", name="w")
        nc.sync.dma_start(out=ost_sb[:, 0:2048], in_=ost_r[:, 0:2048])
        nc.sync.dma_start(out=w_sb[:, 0:2048], in_=w_r[:, 0:2048])
        nc.sync.dma_start(out=ost_sb[:, 2048:4096], in_=ost_r[:, 2048:4096])
        for q in range(1, 4):
            nc.sync.dma_start(out=w_sb[:, q * 2048:(q + 1) * 2048],
                              in_=w_r[:, q * 2048:(q + 1) * 2048])
        ht_sb = []
        for p in range(4):
            t = big.tile([128, 8192], F16, tag=f"ht{p}", name=f"ht{p}")
            nc.sync.dma_start(out=t, in_=ht_r[:, p * 8192:(p + 1) * 8192])
            ht_sb.append(t)

        # ---- Phase A: G.T = (out_state_slice @ W).T, [m, s] layout ----
        # w_sb[:, m*1024 + n*128 + c] = W[n*128 + p, m*128 + c]
        # ost_sb[:, n*512 + s] = out_state_slice[s, n*128 + p]
        gt = big.tile([128, NB_M * SC], F16, tag="gt", name="gt")
        for m in range(NB_M):
            pg = ps.tile([128, SC], F32, tag="ps")
            for n in range(NB_M):
                nc.tensor.matmul(pg[:],
                                 lhsT=w_sb[:, m * N + n * 128:m * N + (n + 1) * 128],
                                 rhs=ost_sb[:, n * SC:(n + 1) * SC],
                                 start=(n == 0), stop=(n == NB_M - 1))
            nc.vector.tensor_copy(out=gt[:, m * SC:(m + 1) * SC], in_=pg[:])

        # ---- Phase B: scores + streaming exp ----
        expb = [big.tile([128, S1], F16, tag=f"exp{i}", name=f"exp{i}")
                for i in range(NB_I)]
        nmax = [small.tile([128, NB_T], F32, tag=f"nmax{i}", name=f"nmax{i}")
                for i in range(NB_I)]
        ssum = [small.tile([128, NB_T], F32, tag=f"ssum{i}", name=f"ssum{i}")
                for i in range(NB_I)]

        def do_block(b, i):
            pair, off = divmod(b, 2)
            off *= 512
            psc = ps.tile([128, 512], F32, tag="ps")
            for m in range(NB_M):
                nc.tensor.matmul(
                    psc[:],
                    lhsT=gt[:, m * SC + i * 128:m * SC + (i + 1) * 128],
                    rhs=ht_sb[pair][:, m * 1024 + off:m * 1024 + off + 512],
                    start=(m == 0), stop=(m == NB_M - 1))
            nc.vector.tensor_reduce(out=nmax[i][:, b:b + 1], in_=psc[:],
                                    axis=mybir.AxisListType.X,
                                    op=mybir.AluOpType.max, negate=True)
            nc.scalar.activation(out=expb[i][:, b * 512:(b + 1) * 512],
                                 in_=psc[:],
                                 func=mybir.ActivationFunctionType.Exp,
                                 bias=nmax[i][:, b:b + 1], scale=1.0,
                                 accum_out=ssum[i][:, b:b + 1])

        def finalize(i):
            """Global max over block maxes, rescale factors, scale+store.

            nmax holds nm_ib = -m_ib; NM_i = min_b nm_ib = -M_i, so
            e_ib = exp(m_ib - M_i) = Exp(in=nm_ib, scale=-1, bias=NM_i).
            """
            nm = small.tile([128, 1], F32, tag=f"nm{i}", name=f"nm{i}")
            nc.vector.tensor_reduce(out=nm[:], in_=nmax[i][:],
                                    axis=mybir.AxisListType.X,
                                    op=mybir.AluOpType.min)
            e = small.tile([128, NB_T], F32, tag=f"e{i}", name=f"e{i}")
            nc.scalar.activation(out=e[:], in_=nmax[i][:],
                                 func=mybir.ActivationFunctionType.Exp,
                                 bias=nm[:], scale=-1.0)
            wsum = small.tile([128, NB_T], F32, tag=f"ws{i}", name=f"ws{i}")
            nc.vector.tensor_mul(wsum[:], e[:], ssum[i][:])
            s = small.tile([128, 1], F32, tag=f"s{i}", name=f"s{i}")
            nc.vector.tensor_reduce(out=s[:], in_=wsum[:],
                                    axis=mybir.AxisListType.X,
                                    op=mybir.AluOpType.add)
            r = small.tile([128, 1], F32, tag=f"r{i}", name=f"r{i}")
            nc.vector.reciprocal(out=r[:], in_=s[:])
            f = small.tile([128, NB_T], F32, tag=f"f{i}", name=f"f{i}")
            nc.vector.tensor_scalar_mul(f[:], e[:], r[:])
            o = out_pool.tile([128, S1], F16, tag=f"out{i % 2}", name=f"out{i}")
            rows = slice(i * 128, (i + 1) * 128)

            def scale(b, eng):
                sl = slice(b * 512, (b + 1) * 512)
                if eng == "v":
                    nc.vector.tensor_scalar_mul(o[:, sl], expb[i][:, sl],
                                                f[:, b:b + 1])
                else:
                    nc.scalar.mul(o[:, sl], expb[i][:, sl], f[:, b:b + 1])

            if i < NB_I - 1:
                # DVE does most blocks (it is cheaper per op); stores in
                # halves on the idle sync ring, overlapped by next chunk.
                for b, eng in [(0, "v"), (1, "v"), (2, "v"), (3, "s")]:
                    scale(b, eng)
                nc.sync.dma_start(out=probs[rows, 0:2048], in_=o[:, 0:2048])
                for b, eng in [(4, "v"), (5, "v"), (6, "v"), (7, "s")]:
                    scale(b, eng)
                nc.sync.dma_start(out=probs[rows, 2048:4096], in_=o[:, 2048:4096])
            else:
                # last chunk: quarter stores alternating rings, DVE-heavy
                ring = [nc.sync, nc.scalar, nc.sync, nc.scalar]
                eng = ["v", "v", "v", "v", "v", "v", "v", "s"]
                for q in range(4):
                    scale(2 * q, eng[2 * q])
                    scale(2 * q + 1, eng[2 * q + 1])
                    ring[q].dma_start(out=probs[rows, q * 1024:(q + 1) * 1024],
                                      in_=o[:, q * 1024:(q + 1) * 1024])

        # part 1: blocks 0..3 block-major (all chunks advance per ht pair)
        for b in range(4):
            for i in range(NB_I):
                do_block(b, i)
        # part 2: blocks 4..7 chunk-major; finalize+store overlap next chunk
        for i in range(NB_I):
            for b in range(4, NB_T):
                do_block(b, i)
            finalize(i)

    nc.finalize()
    return nc


def _get_nc():
    if "nc" not in _CACHE:
        _CACHE["nc"] = _build()
    return _CACHE["nc"]


def kernel(out_state, history, W, b):
    from concourse.bass_utils import run_bass_kernel_spmd

    out_state = np.ascontiguousarray(out_state, dtype=np.float32)
    history = np.ascontiguousarray(history, dtype=np.float32)
    W = np.ascontiguousarray(W, dtype=np.float32)

    # w_r[p, m*1024 + n*128 + c] = W[n*128 + p, m*128 + c]
    w_p = np.ascontiguousarray(
        W.reshape(8, 128, 8, 128).transpose(1, 2, 0, 3).reshape(128, 8192))
    # ht_r[p, pair*8192 + m*1024 + t] = history[pair*1024 + t, m*128 + p]
    ht_p = np.ascontiguousarray(
        history.T.reshape(8, 128, 4, 1024).transpose(1, 2, 0, 3).reshape(128, 32768)
    ).astype(np.float16)

    in_maps = []
    for c in range(NCORES):
        sl = out_state[c * SC:(c + 1) * SC, :]          # [512, 1024]
        # ost_r[p, n*512 + s] = sl[s, n*128 + p]
        ost_p = np.ascontiguousarray(
            sl.T.reshape(8, 128, SC).transpose(1, 0, 2).reshape(128, 4096))
        in_maps.append({"w_r": w_p, "ost_r": ost_p, "ht_r": ht_p})

    nc = _get_nc()
    trace = bool(int(os.environ.get("KERNEL_TRACE", "0")))
    res = run_bass_kernel_spmd(nc, in_maps, list(range(NCORES)), trace=trace)
    _CACHE["last_result"] = res
    out = np.empty((S2, S1), dtype=np.float32)
    for c in range(NCORES):
        out[c * SC:(c + 1) * SC, :] = res.results[c]["probs"].astype(np.float32)
    return out


# revision 9
# speedup vs baseline: 1.0176x; 1.0176x over previous
"""TRN2 Bass kernel for nn_Attn_63230508532520.

reference:
    proj = history @ W.T + b            # [S1, N]
    energies = out_state @ proj.T       # [S2, S1]
    out = softmax(energies, axis=-1)

Math used here:
    energies = out_state @ W @ history.T + (out_state @ b) 1^T
    The bias term is constant per row -> softmax-invariant -> dropped.
    G = out_state @ W (per-core slice), scores = G @ history.T, row softmax.

Sharding: out_state rows (S2=4096) split across 8 cores (512 rows each);
W and history replicated. history.T is fed as fp16 (PE runs fp16 at the
same 1-pass rate as fp32r; halves the dominant HBM stream 16->8 MB/core).
W/out_state stay fp32r (the PE rejects mixed 32/16-bit matmuls, so G is
stored fp16 for the scores matmul). Measured rel err ~1.1e-2 vs 2e-2 gate.

Per-core pipeline (inputs host-packed so each DMA is one large
contiguous-per-partition transfer on the sync HWDGE ring, strict FIFO
order ost/W interleaved -> ht pairs; PE cadence is 216ns/MM = roofline):
  Phase A: G.T [128m, 512s] per m-group = W-panel-stationary fp32r
           matmuls accumulated over n, PSUM evacuated to fp32r SBUF.
  Phase B part 1 (ht col-blocks 0..3): block-major over the first two
           ht pairs; per (block, s-chunk): 8 matmuls into a rotating
           PSUM bank, block max (DVE, negated), exp(x - max) from PSUM
           with accum_out row sums (ACT) into fp16 SBUF.
  Phase B part 2 (blocks 4..7): s-chunk-major so each chunk's softmax
           finalize + output store overlaps the next chunk's matmuls:
           global max over the 8 block maxes, factors
           f_ib = exp(m_ib - M_i)/S_i, scale of the 8 col-blocks split
           across DVE/ACT, stores on the (by then idle) sync ring; the
           last chunk stores quarter-wise across both HWDGE rings to
           shorten the tail.
Output fp16 (rounding 5e-4, far below the matmul noise); host upcasts.
"""
import os
import numpy as np
from contextlib import ExitStack

S2, S1, N = 4096, 4096, 1024
NCORES = 8
SC = S2 // NCORES          # 512 rows per core
NB_M = N // 128            # 8 contraction chunks
NB_I = SC // 128           # 4 s-chunks per core
NB_T = S1 // 512           # 8 t-blocks

_CACHE = {}


def _build():
    import concourse.bacc as bacc
    import concourse.mybir as mybir
    import concourse.tile as tile

    F32 = mybir.dt.float32
    F32R = mybir.dt.float32r
    F16 = mybir.dt.float16

    nc = bacc.Bacc()
    # host-packed layouts (see kernel() below)
    ost_r = nc.declare_dram_parameter("ost_r", [128, NB_M * SC], F32R, isOutput=False)
    w_r = nc.declare_dram_parameter("w_r", [128, NB_M * N], F32R, isOutput=False)
    ht_r = nc.declare_dram_parameter("ht_r", [128, NB_M * S1], F16, isOutput=False)
    probs = nc.declare_dram_parameter("probs", [SC, S1], F16, isOutput=True)

    with tile.TileContext(nc) as tc, ExitStack() as ctx:
        big = ctx.enter_context(tc.tile_pool(name="big", bufs=1))
        out_pool = ctx.enter_context(tc.tile_pool(name="outp", bufs=2))
        small = ctx.enter_context(tc.tile_pool(name="small", bufs=1))
        ps = ctx.enter_context(tc.tile_pool(name="ps", bufs=8, space="PSUM"))

        # ---- input loads: strict FIFO order on the sync ring, 0.5-1 MB
        # pieces interleaved in PE consumption order so Phase A starts as
        # early as possible and the PE is never starved afterwards ----
        ost_sb = big.tile([128, NB_M * SC], F32R, tag="ost", name="ost")
        w_sb = big.tile([128, NB_M * N], F32R, tag="w", name="w")
        nc.sync.dma_start(out=ost_sb[:, 0:1024], in_=ost_r[:, 0:1024])
        nc.sync.dma_start(out=w_sb[:, 0:1024], in_=w_r[:, 0:1024])
        nc.sync.dma_start(out=ost_sb[:, 1024:2048], in_=ost_r[:, 1024:2048])
        nc.sync.dma_start(out=w_sb[:, 1024:2048], in_=w_r[:, 1024:2048])
        nc.sync.dma_start(out=ost_sb[:, 2048:3072], in_=ost_r[:, 2048:3072])
        nc.sync.dma_start(out=ost_sb[:, 3072:4096], in_=ost_r[:, 3072:4096])
        for m in range(2, NB_M):
            nc.sync.dma_start(out=w_sb[:, m * 1024:(m + 1) * 1024],
                              in_=w_r[:, m * 1024:(m + 1) * 1024])
        ht_sb = []
        for bb in range(NB_T):
            t = big.tile([128, 4096], F16, tag=f"ht{bb}", name=f"ht{bb}")
            nc.sync.dma_start(out=t, in_=ht_r[:, bb * 4096:(bb + 1) * 4096])
            ht_sb.append(t)

        # ---- Phase A: G.T = (out_state_slice @ W).T, [m, s] layout ----
        # w_sb[:, m*1024 + n*128 + c] = W[n*128 + p, m*128 + c]
        # ost_sb[:, n*512 + s] = out_state_slice[s, n*128 + p]
        gt = big.tile([128, NB_M * SC], F16, tag="gt", name="gt")
        for m in range(NB_M):
            pg = ps.tile([128, SC], F32, tag="ps")
            for n in range(NB_M):
                nc.tensor.matmul(pg[:],
                                 lhsT=w_sb[:, m * N + n * 128:m * N + (n + 1) * 128],
                                 rhs=ost_sb[:, n * SC:(n + 1) * SC],
                                 start=(n == 0), stop=(n == NB_M - 1))
            nc.vector.tensor_copy(out=gt[:, m * SC:(m + 1) * SC], in_=pg[:])

        # ---- Phase B: scores + streaming exp ----
        expb = [big.tile([128, S1], F16, tag=f"exp{i}", name=f"exp{i}")
                for i in range(NB_I)]
        nmax = [small.tile([128, NB_T], F32, tag=f"nmax{i}", name=f"nmax{i}")
                for i in range(NB_I)]
        ssum = [small.tile([128, NB_T], F32, tag=f"ssum{i}", name=f"ssum{i}")
                for i in range(NB_I)]

        def do_block(b, i):
            psc = ps.tile([128, 512], F32, tag="ps")
            for m in range(NB_M):
                nc.tensor.matmul(
                    psc[:],
                    lhsT=gt[:, m * SC + i * 128:m * SC + (i + 1) * 128],
                    rhs=ht_sb[b][:, m * 512:(m + 1) * 512],
                    start=(m == 0), stop=(m == NB_M - 1))
            nc.vector.tensor_reduce(out=nmax[i][:, b:b + 1], in_=psc[:],
                                    axis=mybir.AxisListType.X,
                                    op=mybir.AluOpType.max, negate=True)
            nc.scalar.activation(out=expb[i][:, b * 512:(b + 1) * 512],
                                 in_=psc[:],
                                 func=mybir.ActivationFunctionType.Exp,
                                 bias=nmax[i][:, b:b + 1], scale=1.0,
                                 accum_out=ssum[i][:, b:b + 1])

        def finalize(i):
            """Global max over block maxes, rescale factors, scale+store.

            nmax holds nm_ib = -m_ib; NM_i = min_b nm_ib = -M_i, so
            e_ib = exp(m_ib - M_i) = Exp(in=nm_ib, scale=-1, bias=NM_i).
            """
            nm = small.tile([128, 1], F32, tag=f"nm{i}", name=f"nm{i}")
            nc.vector.tensor_reduce(out=nm[:], in_=nmax[i][:],
                                    axis=mybir.AxisListType.X,
                                    op=mybir.AluOpType.min)
            e = small.tile([128, NB_T], F32, tag=f"e{i}", name=f"e{i}")
            nc.scalar.activation(out=e[:], in_=nmax[i][:],
                                 func=mybir.ActivationFunctionType.Exp,
                                 bias=nm[:], scale=-1.0)
            wsum = small.tile([128, NB_T], F32, tag=f"ws{i}", name=f"ws{i}")
            nc.vector.tensor_mul(wsum[:], e[:], ssum[i][:])
            s = small.tile([128, 1], F32, tag=f"s{i}", name=f"s{i}")
            nc.vector.tensor_reduce(out=s[:], in_=wsum[:],
                                    axis=mybir.AxisListType.X,
                                    op=mybir.AluOpType.add)
            r = small.tile([128, 1], F32, tag=f"r{i}", name=f"r{i}")
            nc.vector.reciprocal(out=r[:], in_=s[:])
            f = small.tile([128, NB_T], F32, tag=f"f{i}", name=f"f{i}")
            nc.vector.tensor_scalar_mul(f[:], e[:], r[:])
            o = out_pool.tile([128, S1], F16, tag=f"out{i % 2}", name=f"out{i}")
            rows = slice(i * 128, (i + 1) * 128)

            def scale(b, eng):
                sl = slice(b * 512, (b + 1) * 512)
                if eng == "v":
                    nc.vector.tensor_scalar_mul(o[:, sl], expb[i][:, sl],
                                                f[:, b:b + 1])
                else:
                    nc.scalar.mul(o[:, sl], expb[i][:, sl], f[:, b:b + 1])

            if i < NB_I - 1:
                # DVE does most blocks (it is cheaper per op); stores in
                # halves on the idle sync ring, overlapped by next chunk.
                for b, eng in [(0, "v"), (1, "v"), (2, "v"), (3, "s")]:
                    scale(b, eng)
                nc.sync.dma_start(out=probs[rows, 0:2048], in_=o[:, 0:2048])
                for b, eng in [(4, "v"), (5, "v"), (6, "v"), (7, "s")]:
                    scale(b, eng)
                nc.sync.dma_start(out=probs[rows, 2048:4096], in_=o[:, 2048:4096])
            else:
                # last chunk: quarter stores alternating rings, DVE-heavy
                ring = [nc.sync, nc.scalar, nc.sync, nc.scalar]
                eng = ["v", "v", "v", "v", "v", "v", "v", "s"]
                for q in range(4):
                    scale(2 * q, eng[2 * q])
                    scale(2 * q + 1, eng[2 * q + 1])
                    ring[q].dma_start(out=probs[rows, q * 1024:(q + 1) * 1024],
                                      in_=o[:, q * 1024:(q + 1) * 1024])

        # part 1: blocks 0..3 block-major (all chunks advance per ht pair)
        for b in range(4):
            for i in range(NB_I):
                do_block(b, i)
        # part 2: blocks 4..7 chunk-major; finalize+store overlap next chunk
        for i in range(NB_I):
            for b in range(4, NB_T):
                do_block(b, i)
            finalize(i)

    nc.finalize()
    return nc


def _get_nc():
    if "nc" not in _CACHE:
        _CACHE["nc"] = _build()
    return _CACHE["nc"]


def kernel(out_state, history, W, b):
    from concourse.bass_utils import run_bass_kernel_spmd

    out_state = np.ascontiguousarray(out_state, dtype=np.float32)
    history = np.ascontiguousarray(history, dtype=np.float32)
    W = np.ascontiguousarray(W, dtype=np.float32)

    # w_r[p, m*1024 + n*128 + c] = W[n*128 + p, m*128 + c]
    w_p = np.ascontiguousarray(
        W.reshape(8, 128, 8, 128).transpose(1, 2, 0, 3).reshape(128, 8192))
    # ht_r[p, b*4096 + m*512 + t] = history[b*512 + t, m*128 + p]
    ht_p = np.ascontiguousarray(
        history.T.reshape(8, 128, 8, 512).transpose(1, 2, 0, 3).reshape(128, 32768)
    ).astype(np.float16)

    in_maps = []
    for c in range(NCORES):
        sl = out_state[c * SC:(c + 1) * SC, :]          # [512, 1024]
        # ost_r[p, n*512 + s] = sl[s, n*128 + p]
        ost_p = np.ascontiguousarray(
            sl.T.reshape(8, 128, SC).transpose(1, 0, 2).reshape(128, 4096))
        in_maps.append({"w_r": w_p, "ost_r": ost_p, "ht_r": ht_p})

    nc = _get_nc()
    trace = bool(int(os.environ.get("KERNEL_TRACE", "0")))
    res = run_bass_kernel_spmd(nc, in_maps, list(range(NCORES)), trace=trace)
    _CACHE["last_result"] = res
    out = np.empty((S2, S1), dtype=np.float32)
    for c in range(NCORES):
        out[c * SC:(c + 1) * SC, :] = res.results[c]["probs"].astype(np.float32)
    return out
